# revision 28
# baseline (speedup 1.0000x reference)
"""Trainium2 Bass kernel for nn_Attention_11338713661523 (dense transformer
joint-attention block, B=1, S=512+1536, D=3072, H=24, HD=128).

Strategy (tensor-parallel over heads, 8 cores, 3 heads each):
 - Host folds the SVD low-rank branches into the dense weights
   (W_eff = W.T + down @ up), concatenates txt+img tokens, pre-transposes
   X, and builds per-token RoPE/norm tables (norm weights, 1/sqrt(HD) and
   the rope pair-swap signs folded in).
 - Device per core: QKV projection ([tok, feat] layout, fp16 matmuls,
   fp32 psum), RMSNorm via ACT Square+accum / Sqrt + DVE reciprocal,
   RoPE via 3 DVE tensor-tensor ops, PE transpose of q/k to [hd, tok],
   S^T = k_tile^T-stationary matmul per 128-chunk of keys, exp on ACT,
   PV accumulation with v as stationary, softmax denominator via a DVE
   pairwise tree + ones-matmul partition reduction, out-projection
   (row-parallel) into fp16 partial outputs.
 - Host sums the 8 partial outputs and adds biases.
"""

import sys

if '/opt/trn_rl_repo' not in sys.path:
    sys.path.insert(0, '/opt/trn_rl_repo')

from contextlib import ExitStack

import numpy as np

import concourse.bass as bass
import concourse.mybir as mybir
import concourse.tile as tile
from concourse.bass import ts
from concourse.vector_clock import ScopedClock

P = 128
S = 2048
S_TXT = 512
D = 3072
NT = S // P            # 16 token tiles
ND = D // P            # 24 contraction chunks
HL = 3                 # heads per core
HD = 128
FQ = HL * HD           # 384 local q (or k, or v) features
NCORES = 8
EPS = 1e-5

F16 = mybir.dt.float16
F32 = mybir.dt.float32
AF = mybir.ActivationFunctionType
ALU = mybir.AluOpType


def _patch_tile_drain():
    """This walrus build allows only one sync-wait on a Drain CTRL
    instruction; split the final TileContext drain's waits across
    single-wait nops."""
    if getattr(tile.TileContext, '_drain_patched', False):
        return

    def _drain_and_barrier(self, tick_clock, wait_clock):
        nc = self.nc
        drain_inst = nc.sync.drain()
        wait_clock.add_sem_waits(
            drain_inst.ins, ScopedClock({None: tick_clock.global_clock})
        )
        si = drain_inst.ins.sync_info
        waits = list(si.on_wait or [])
        if len(waits) > 1:
            si.on_wait = waits[:1]
            for w in waits[1:]:
                nop = nc.sync.nop(nofuse=True)
                nop.ins.sync_info = mybir.SyncInfo(on_wait=[w], on_update=[])
        nc.all_engine_barrier()
        assert self.sems is not None
        popped = nc._tile_sem_poison_stack.pop()
        assert popped is self._sem_poison
        nc.clear_and_free_semaphores(list(self.sems.allocated().values()))
        nc.all_engine_barrier()

    tile.TileContext._drain_and_barrier = _drain_and_barrier
    tile.TileContext._drain_patched = True


def _split_excess_waits(nc, cap=1):
    """This walrus build supports very few sync-wait slots per instruction.
    Hoist excess waits onto preceding single-wait nops on the same engine."""
    uid = [0]
    for fn in nc.m.functions:
        for bb in fn.blocks:
            out = []
            changed = False
            for inst in bb.instructions:
                si = getattr(inst, 'sync_info', None)
                waits = list(si.on_wait) if si is not None and si.on_wait else []
                if len(waits) > cap and inst.engine is not None:
                    changed = True
                    for w in waits[:-cap]:
                        uid[0] += 1
                        out.append(mybir.InstNoOp(
                            name=f"wsplit_{uid[0]}",
                            sync_info=mybir.SyncInfo(on_wait=[w], on_update=[]),
                            bass_nofuse=True,
                            engine=inst.engine,
                        ))
                    si.on_wait = waits[-cap:]
                out.append(inst)
            if changed:
                bb.instructions = out


def _swap_pairs(ap):
    """View of [..., 2k] AP with adjacent element pairs swapped; returns a
    4-d [P, HL, HD//2, 2] view."""
    return ap.rearrange("p h (a b) -> p h a b", b=2)[:, :, :, ::-1]


def _pairs(ap):
    return ap.rearrange("p h (a b) -> p h a b", b=2)


def build_bass():
    _patch_tile_drain()
    nc = bass.Bass()

    # DRAM I/O (all pre-laid-out on host to match SBUF tiles).
    xt = nc.dram_tensor("xt", [NT, P, ND, P], F16, kind="ExternalInput")
    wqkv_t = nc.dram_tensor("wqkv_t", [P, ND, 3 * FQ], F16, kind="ExternalInput")
    wqkv_i = nc.dram_tensor("wqkv_i", [P, ND, 3 * FQ], F16, kind="ExternalInput")
    wout_t = nc.dram_tensor("wout_t", [P, HL, D], F16, kind="ExternalInput")
    wout_i = nc.dram_tensor("wout_i", [P, HL, D], F16, kind="ExternalInput")
    caq = nc.dram_tensor("caq", [P, NT, HD], F16, kind="ExternalInput")
    saq = nc.dram_tensor("saq", [P, NT, HD], F16, kind="ExternalInput")
    cak = nc.dram_tensor("cak", [P, NT, HD], F16, kind="ExternalInput")
    sak = nc.dram_tensor("sak", [P, NT, HD], F16, kind="ExternalInput")
    ident = nc.dram_tensor("ident", [P, P], F16, kind="ExternalInput")
    ones_col = nc.dram_tensor("ones_col", [P, 1], F16, kind="ExternalInput")
    ones_row = nc.dram_tensor("ones_row", [1, P], F16, kind="ExternalInput")
    epsb = nc.dram_tensor("epsb", [P, 1], F32, kind="ExternalInput")
    y = nc.dram_tensor("y", [S, D], F16, kind="ExternalOutput")

    with tile.TileContext(nc) as tc, ExitStack() as ctx:
        const = ctx.enter_context(tc.tile_pool(name="const", bufs=1))
        persist = ctx.enter_context(tc.tile_pool(name="persist", bufs=1))
        np2 = ctx.enter_context(tc.tile_pool(name="np2", bufs=2))

        ident_sb = const.tile([P, P], F16, tag="ident")
        onec_sb = const.tile([P, 1], F16, tag="onec")
        oner_sb = const.tile([1, P], F16, tag="oner")
        eps_sb = const.tile([P, 1], F32, tag="epsb")
        tabs = {}
        for name in ("caq", "saq", "cak", "sak"):
            tabs[name] = const.tile([P, NT, HD], F16, tag=name, name=name)

        # persistent activations; attnT split per q-tile group so the out
        # projection can start as soon as a group's softmax is finalized
        qT = persist.tile([P, HL, S], F16, tag="qT")
        kT = persist.tile([P, HL, S], F16, tag="kT")
        v_sb = persist.tile([P, NT, FQ], F16, tag="v")
        NQT = 4
        QW = S // NQT    # 512
        attnT = [persist.tile([P, HL, QW], F16, tag=f"attnT{g}",
                              name=f"attnT{g}")
                 for g in range(NQT)]
        # tile-15 q/k psums are evacuated to SBUF so their norm/rope/transpose
        # pipelines can run inside phase 2 (removes the phase-1 tail)
        q15_sb = persist.tile([P, FQ], F32, tag="q15")
        k15_sb = persist.tile([P, FQ], F32, tag="k15")

        def qk_pipeline(src3, ca_t, sa_t, dstT, i, ps_pool, ps_tag):
            """RMSNorm + RoPE + PE-transpose for one [P, HL, HD] q/k tile.
            src3 may live in PSUM or SBUF (fp32)."""
            ms = np2.tile([P, HL], F32, tag="ms")
            for h in range(HL):
                scr = np2.tile([P, HD], F32, tag="sqscr")
                nc.scalar.activation(scr[:], src3[:, h, :], AF.Square,
                                     accum_out=ms[:, h:h + 1])
            srt = np2.tile([P, HL], F32, tag="srt")
            nc.scalar.activation(srt[:], ms[:], AF.Sqrt,
                                 scale=1.0 / HD, bias=eps_sb[:])
            s = np2.tile([P, HL], F32, tag="s")
            nc.vector.reciprocal(s[:], srt[:])
            qn = np2.tile([P, HL, HD], F16, tag="qn")
            for h in range(HL):
                nc.scalar.mul(qn[:, h, :], src3[:, h, :], s[:, h:h + 1])
            # rope: o = qn*ca + swap(qn)*sa
            ca_b = ca_t[:, i, :].unsqueeze(1).to_broadcast([P, HL, HD])
            sa_b = sa_t[:, i, :].unsqueeze(1).to_broadcast([P, HL, HD])
            ra = np2.tile([P, HL, HD], F16, tag="ra")
            nc.vector.tensor_tensor(ra[:], qn[:], ca_b, ALU.mult)
            rb = np2.tile([P, HL, HD], F16, tag="rb")
            nc.vector.tensor_tensor(_pairs(rb), _swap_pairs(qn),
                                    _pairs(sa_b), ALU.mult)
            qr = np2.tile([P, HL, HD], F16, tag="qr")
            nc.vector.tensor_tensor(qr[:], ra[:], rb[:], ALU.add)
            for h in range(HL):
                ps_t = ps_pool.tile([P, P], F32, tag=ps_tag,
                                    name=f"tr_{i}_{h}")
                nc.tensor.matmul(ps_t[:], lhsT=qr[:, h, :], rhs=ident_sb[:],
                                 start=True, stop=True)
                nc.vector.tensor_copy(dstT[:, h, ts(i, P)], ps_t[:])

        # ---------------- Phase 1: QKV + norm + rope + transpose ----------
        with tc.tile_pool(name="wqkv", bufs=1) as wpool, \
             tc.tile_pool(name="xp", bufs=4) as xpool, \
             tc.tile_pool(name="ps_qkv", bufs=2, space="PSUM") as psum_qkv, \
             tc.tile_pool(name="ps_tr", bufs=2, space="PSUM") as psum_tr:

            # DMA emission order tuned for the serial ~0.6us/trigger on the
            # sync sequencer and ~400 GB/s aggregate bandwidth: xt tile 0,
            # first weight chunks, rope tables mid-stream, remaining weights
            w_txt_sb = wpool.tile([P, ND, 3 * FQ], F16, tag="wt")
            w_img_sb = wpool.tile([P, ND, 3 * FQ], F16, tag="wi")

            def load_xt(i):
                strip = xpool.tile([P, ND, P], F16, tag="xt", name=f"xt{i}")
                nc.sync.dma_start(strip[:], xt[i])
                return strip

            xt_strips = [None] * NT
            xt_strips[0] = load_xt(0)
            for j in range(0, 4):
                nc.sync.dma_start(w_txt_sb[:, j, :], wqkv_t[:, j, :])
            for j in range(4, 12, 2):
                nc.sync.dma_start(w_txt_sb[:, j:j + 2, :], wqkv_t[:, j:j + 2, :])
            for name, dram in (("caq", caq), ("saq", saq),
                               ("cak", cak), ("sak", sak)):
                nc.sync.dma_start(tabs[name][:], dram[:])
            for j in range(12, ND, 2):
                nc.sync.dma_start(w_txt_sb[:, j:j + 2, :], wqkv_t[:, j:j + 2, :])
            nc.sync.dma_start(ident_sb[:], ident[:])
            nc.sync.dma_start(onec_sb[:], ones_col[:])
            nc.sync.dma_start(oner_sb[:], ones_row[:])
            nc.sync.dma_start(eps_sb[:], epsb[:])
            for i in range(1, 4):
                xt_strips[i] = load_xt(i)
            for j in range(0, ND, 2):
                nc.sync.dma_start(w_img_sb[:, j:j + 2, :], wqkv_i[:, j:j + 2, :])

            for i in range(NT):
                w_sb = w_txt_sb if i < S_TXT // P else w_img_sb
                if xt_strips[i] is None:
                    xt_strips[i] = load_xt(i)
                if i + 3 < NT and xt_strips[i + 3] is None:
                    xt_strips[i + 3] = load_xt(i + 3)
                xt_strip = xt_strips[i]

                psq = psum_qkv.tile([P, FQ], F32, tag="psq")
                psk = psum_qkv.tile([P, FQ], F32, tag="psk")
                psv = psum_qkv.tile([P, FQ], F32, tag="psv")
                for j in range(ND):
                    lh = xt_strip[:, j, :]
                    st = (j == 0)
                    sp = (j == ND - 1)
                    nc.tensor.matmul(psq[:], lhsT=lh, rhs=w_sb[:, j, 0:FQ],
                                     start=st, stop=sp)
                    nc.tensor.matmul(psk[:], lhsT=lh, rhs=w_sb[:, j, FQ:2 * FQ],
                                     start=st, stop=sp)
                    nc.tensor.matmul(psv[:], lhsT=lh, rhs=w_sb[:, j, 2 * FQ:3 * FQ],
                                     start=st, stop=sp)

                nc.vector.tensor_copy(v_sb[:, i, :], psv[:])

                if i == NT - 1:
                    # defer tile-15 q/k processing into phase 2
                    nc.vector.tensor_copy(q15_sb[:], psq[:])
                    nc.vector.tensor_copy(k15_sb[:], psk[:])
                else:
                    qk_pipeline(psq.rearrange("p (h d) -> p h d", d=HD),
                                tabs["caq"], tabs["saq"], qT, i,
                                psum_tr, "tr")
                    qk_pipeline(psk.rearrange("p (h d) -> p h d", d=HD),
                                tabs["cak"], tabs["sak"], kT, i,
                                psum_tr, "tr")

        # ---------------- Phase 2: attention -----------------------------
        # qt-outer; denominator finalize staged at hook points one iteration
        # later so no in-order engine queue blocks. Out-projection weights
        # prefetched here (phase-1 weight space was just freed).
        wopool = ctx.enter_context(tc.tile_pool(name="wout", bufs=1))
        wo_txt_sb = wopool.tile([P, HL, D], F16, tag="wot")
        wo_img_sb = wopool.tile([P, HL, D], F16, tag="woi")
        for ch in range(HL):
            nc.sync.dma_start(wo_txt_sb[:, ch, :], wout_t[:, ch, :])
        for ch in range(HL):
            nc.sync.dma_start(wo_img_sb[:, ch, :], wout_i[:, ch, :])

        with tc.tile_pool(name="pp", bufs=6) as ppool, \
             tc.tile_pool(name="tree", bufs=24) as tpool, \
             tc.tile_pool(name="rp", bufs=2) as rpool, \
             tc.tile_pool(name="ps_s", bufs=2, space="PSUM") as psum_s, \
             tc.tile_pool(name="ps_u", bufs=2, space="PSUM") as psum_u, \
             tc.tile_pool(name="ps_d", bufs=1, space="PSUM") as psum_d, \
             tc.tile_pool(name="ps_r", bufs=1, space="PSUM") as psum_r:

            def attn_compute(h, qt, hooks=()):
                # hooks: {ck2_group: callable} run after that group's emission
                ps_u = psum_u.tile([P, QW], F32, tag="u")
                lvl = []
                hooks = dict(hooks)
                for ck2 in range(NT // 2):
                    ps_s = psum_s.tile([P, 2 * QW], F32, tag="s")
                    for half in range(2):
                        ck = 2 * ck2 + half
                        nc.tensor.matmul(
                            ps_s[:, half * QW:(half + 1) * QW],
                            lhsT=kT[:, h, ts(ck, P)],
                            rhs=qT[:, h, ts(qt, QW)],
                            start=True, stop=True)
                    p2 = ppool.tile([P, 2 * QW], F16, tag="p2")
                    nc.scalar.activation(p2[:], ps_s[:], AF.Exp)
                    for half in range(2):
                        ck = 2 * ck2 + half
                        nc.tensor.matmul(
                            ps_u[:],
                            lhsT=v_sb[:, ck, ts(h, HD)],
                            rhs=p2[:, half * QW:(half + 1) * QW],
                            start=(ck == 0), stop=(ck == NT - 1))
                    t = tpool.tile([P, QW], F16, tag="tree")
                    # pair-sum each exp tile's halves (alternating DVE/GpSimd,
                    # last group on DVE so the tail chain stays short), then
                    # fold into a running sum on DVE
                    eng = nc.vector if (ck2 % 2 == 0 or ck2 == NT // 2 - 1) \
                        else nc.gpsimd
                    eng.tensor_tensor(t[:], p2[:, 0:QW], p2[:, QW:2 * QW],
                                      ALU.add)
                    if lvl:
                        r = tpool.tile([P, QW], F16, tag="tree")
                        nc.vector.tensor_tensor(r[:], lvl[-1][:], t[:], ALU.add)
                        lvl.append(r)
                    else:
                        lvl.append(t)
                    if ck2 in hooks:
                        hooks[ck2]()
                return ps_u, lvl[-1]

            # denominator finalize for iteration k-1, staged inside iteration
            # k: d-matmul, then 1/d = exp(-ln d) on ACT (same table set as the
            # softmax exp), then the broadcast matmul + normalizing multiply
            def make_hooks(prev):
                ph, pqt, pps_u, pdsum = prev
                st = {}

                def s0():
                    ps_d = psum_d.tile([1, QW], F32, tag="d")
                    nc.tensor.matmul(ps_d[:], lhsT=onec_sb[:], rhs=pdsum[:],
                                     start=True, stop=True)
                    st['ps_d'] = ps_d

                def s1():
                    lnd = rpool.tile([1, QW], F32, tag="lnd")
                    nc.scalar.activation(lnd[:], st['ps_d'][:], AF.Ln)
                    rcp16 = rpool.tile([1, QW], F16, tag="rcp16")
                    nc.scalar.activation(rcp16[:], lnd[:], AF.Exp, scale=-1.0)
                    st['rcp16'] = rcp16

                def s2():
                    ps_rep = psum_r.tile([P, QW], F32, tag="rep")
                    nc.tensor.matmul(ps_rep[:], lhsT=oner_sb[:],
                                     rhs=st['rcp16'][:], start=True, stop=True)
                    rep16 = rpool.tile([P, QW], F16, tag="rep16")
                    nc.vector.tensor_copy(rep16[:], ps_rep[:])
                    nc.vector.tensor_tensor(attnT[pqt][:, ph, :], pps_u[:],
                                            rep16[:], ALU.mult)

                return {1: s0, 2: s1, 4: s2}

            prev = None
            for qt in range(NQT):
                for h in range(HL):
                    if prev is None:
                        # first iteration: its hook slots instead emit the
                        # deferred tile-15 k/q pipelines (k15 is needed by
                        # this very iteration's last chunk group)
                        hooks = {
                            1: lambda: qk_pipeline(
                                k15_sb.rearrange("p (h d) -> p h d", d=HD),
                                tabs["cak"], tabs["sak"], kT, NT - 1,
                                psum_d, "d"),
                            5: lambda: qk_pipeline(
                                q15_sb.rearrange("p (h d) -> p h d", d=HD),
                                tabs["caq"], tabs["saq"], qT, NT - 1,
                                psum_d, "d"),
                        }
                    else:
                        hooks = make_hooks(prev)
                    prev = (h, qt, *attn_compute(h, qt, hooks))
            # evacuate the last iteration's psum/tree results to SBUF so the
            # tail finalize can run in phase 3 after these pools close
            th, tqt, tps_u, tdsum = prev
            u_tail_sb = wopool.tile([P, QW], F32, tag="u_tail")
            nc.vector.tensor_copy(u_tail_sb[:], tps_u[:])
            dsum_tail_sb = wopool.tile([P, QW], F16, tag="dsum_tail")
            nc.vector.tensor_copy(dsum_tail_sb[:], tdsum[:])
            tail = (th, tqt, u_tail_sb, dsum_tail_sb)

        # ---------------- Phase 3: out projection -------------------------
        YW = 1536
        with tc.tile_pool(name="yp", bufs=3) as ypool, \
             tc.tile_pool(name="ps_y", bufs=2, space="PSUM") as psum_y:
            th, tqt, tps_u, tdsum = tail
            tst = {}

            def tail_s0():
                ps_d = psum_y.tile([1, QW], F32, tag="y", name="tail_ps_d")
                nc.tensor.matmul(ps_d[:], lhsT=onec_sb[:], rhs=tdsum[:],
                                 start=True, stop=True)
                lnd = ypool.tile([1, QW], F32, tag="lnd_t", name="tail_lnd")
                nc.scalar.activation(lnd[:], ps_d[:], AF.Ln)
                rcp16 = ypool.tile([1, QW], F16, tag="rcp16_t",
                                    name="tail_rcp16")
                nc.scalar.activation(rcp16[:], lnd[:], AF.Exp, scale=-1.0)
                tst['rcp16'] = rcp16

            def tail_s1():
                ps_rep = psum_y.tile([P, QW], F32, tag="y", name="tail_ps_rep")
                nc.tensor.matmul(ps_rep[:], lhsT=oner_sb[:],
                                 rhs=tst['rcp16'][:], start=True, stop=True)
                rep16 = ypool.tile([P, QW], F16, tag="rep16_t",
                                    name="tail_rep16")
                nc.vector.tensor_copy(rep16[:], ps_rep[:])
                nc.vector.tensor_tensor(attnT[tqt][:, th, :], tps_u[:],
                                        rep16[:], ALU.mult)

            for i in range(NT):
                w_sb = wo_txt_sb if i < S_TXT // P else wo_img_sb
                g, r = i // (QW // P), i % (QW // P)
                for half in range(2):
                    ps_y = psum_y.tile([P, YW], F32, tag="y")
                    for ch in range(HL):
                        lh = attnT[g][:, ch, ts(r, P)]
                        for dd in range(3):
                            nc.tensor.matmul(
                                ps_y[:, dd * QW:(dd + 1) * QW],
                                lhsT=lh,
                                rhs=w_sb[:, ch, half * YW + dd * QW:
                                         half * YW + (dd + 1) * QW],
                                start=(ch == 0), stop=(ch == HL - 1))
                    y_t = ypool.tile([P, YW], F16, tag="yt")
                    nc.scalar.copy(y_t[:], ps_y[:])
                    nc.sync.dma_start(
                        y[ts(i, P), half * YW:(half + 1) * YW], y_t[:])
                # the last attention iteration's finalize is interleaved here
                # (tile 12-15 are the only consumers of its attnT group)
                if i == 0:
                    tail_s0()
                elif i == 1:
                    tail_s1()

    _split_excess_waits(nc)
    return nc


def _host_prep(inputs):
    f = lambda n: np.asarray(inputs[n], dtype=np.float32)
    hs = f('hidden_states')[0]
    ehs = f('encoder_hidden_states')[0]
    X = np.concatenate([ehs, hs], axis=0)              # [2048, 3072] txt|img
    XT = np.ascontiguousarray(X.T)                     # [3072, 2048]
    # [NT, P(tok), ND, P(d)] tiled layout matching SBUF strips
    xt_tiled = np.ascontiguousarray(
        XT.reshape(ND, P, NT, P).transpose(2, 1, 0, 3)).astype(np.float16)

    Wqkv_i = f('qkv_w').T + f('qkv_down') @ f('qkv_up')          # [3072, 9216]
    Wqkv_t = f('add_qkv_w').T + f('add_qkv_down') @ f('add_qkv_up')
    Wo_i = f('out_w').T + f('out_down') @ f('out_up')            # [3072, 3072]
    Wo_t = f('add_out_w').T + f('add_out_down') @ f('add_out_up')

    cos, sin = f('rope_cos'), f('rope_sin')            # [2048, 64]
    C2 = np.repeat(cos, 2, axis=1)                     # [2048, 128]
    S2 = np.repeat(sin, 2, axis=1)
    S2[:, 0::2] *= -1.0

    def tok_norm_w(w_img, w_txt):
        w = np.empty((S, HD), np.float32)
        w[:S_TXT] = w_txt
        w[S_TXT:] = w_img
        return w

    def swap(w):
        o = np.empty_like(w)
        o[:, 0::2] = w[:, 1::2]
        o[:, 1::2] = w[:, 0::2]
        return o

    wq = tok_norm_w(f('norm_q_w'), f('norm_added_q_w'))
    wk = tok_norm_w(f('norm_k_w'), f('norm_added_k_w'))
    qscale = 1.0 / np.sqrt(np.float32(HD))
    CA_q = (wq * C2) * qscale
    SA_q = (swap(wq) * S2) * qscale
    CA_k = wk * C2
    SA_k = swap(wk) * S2

    def tab16(t):  # [2048, 128] -> [P, NT, HD]
        return np.ascontiguousarray(
            t.reshape(NT, P, HD).transpose(1, 0, 2)).astype(np.float16)

    def wqkv16(W):  # [3072, 9216] core slice -> [P, ND, 3*FQ]
        def core_slice(W, c):
            qs = slice(c * FQ, (c + 1) * FQ)
            ks = slice(D + c * FQ, D + (c + 1) * FQ)
            vs = slice(2 * D + c * FQ, 2 * D + (c + 1) * FQ)
            return np.concatenate([W[:, qs], W[:, ks], W[:, vs]], axis=1)
        return [np.ascontiguousarray(
            core_slice(W, c).reshape(ND, P, 3 * FQ).transpose(1, 0, 2)
        ).astype(np.float16) for c in range(NCORES)]

    def wout16(W):  # [3072, 3072] row slice per core -> [P, HL, D]
        return [np.ascontiguousarray(
            W[c * FQ:(c + 1) * FQ, :].reshape(HL, P, D).transpose(1, 0, 2)
        ).astype(np.float16) for c in range(NCORES)]

    shared = {
        'xt': xt_tiled,
        'epsb': np.full((P, 1), EPS, np.float32),
        'caq': tab16(CA_q), 'saq': tab16(SA_q),
        'cak': tab16(CA_k), 'sak': tab16(SA_k),
        'ident': np.eye(P, dtype=np.float16),
        'ones_col': np.ones((P, 1), np.float16),
        'ones_row': np.ones((1, P), np.float16),
    }
    wqkv_i_c = wqkv16(Wqkv_i)
    wqkv_t_c = wqkv16(Wqkv_t)
    wo_i_c = wout16(Wo_i)
    wo_t_c = wout16(Wo_t)
    in_maps = []
    for c in range(NCORES):
        m = dict(shared)
        m['wqkv_i'] = wqkv_i_c[c]
        m['wqkv_t'] = wqkv_t_c[c]
        m['wout_i'] = wo_i_c[c]
        m['wout_t'] = wo_t_c[c]
        in_maps.append(m)
    return in_maps


_NC_CACHE = None


def run_device(inputs, trace=False, **kw):
    global _NC_CACHE
    from concourse.bass_utils import run_bass_kernel_spmd
    if _NC_CACHE is None:
        _NC_CACHE = build_bass()
    in_maps = _host_prep(inputs)
    return run_bass_kernel_spmd(_NC_CACHE, in_maps, core_ids=list(range(NCORES)),
                                trace=trace, **kw)


def kernel(**inputs):
    res = run_device(inputs, trace=False)
    Y = np.zeros((S, D), np.float32)
    for r in res.results:
        Y += r['y'].astype(np.float32)
    out_b = np.asarray(inputs['out_b'], dtype=np.float32)
    add_out_b = np.asarray(inputs['add_out_b'], dtype=np.float32)
    img = (Y[S_TXT:] + out_b)[None].astype(np.float32)
    txt = (Y[:S_TXT] + add_out_b)[None].astype(np.float32)
    return (img, txt)


# revision 30
# speedup vs baseline: 1.0263x; 1.0263x over previous
"""Trainium2 Bass kernel for nn_Attention_11338713661523 (dense transformer
joint-attention block, B=1, S=512+1536, D=3072, H=24, HD=128).

Strategy (tensor-parallel over heads, 8 cores, 3 heads each):
 - Host folds the SVD low-rank branches into the dense weights
   (W_eff = W.T + down @ up), concatenates txt+img tokens, pre-transposes
   X, and builds per-token RoPE/norm tables (norm weights, 1/sqrt(HD) and
   the rope pair-swap signs folded in).
 - Device per core: QKV projection ([tok, feat] layout, fp16 matmuls,
   fp32 psum), RMSNorm via ACT Square+accum / Sqrt + DVE reciprocal,
   RoPE via 3 DVE tensor-tensor ops, PE transpose of q/k to [hd, tok],
   S^T = k_tile^T-stationary matmul per 128-chunk of keys, exp on ACT,
   PV accumulation with v as stationary, softmax denominator via a DVE
   pairwise tree + ones-matmul partition reduction, out-projection
   (row-parallel) into fp16 partial outputs.
 - Host sums the 8 partial outputs and adds biases.
"""

import sys

if '/opt/trn_rl_repo' not in sys.path:
    sys.path.insert(0, '/opt/trn_rl_repo')

from contextlib import ExitStack

import numpy as np

import concourse.bass as bass
import concourse.mybir as mybir
import concourse.tile as tile
from concourse.bass import ts
from concourse.vector_clock import ScopedClock

P = 128
S = 2048
S_TXT = 512
D = 3072
NT = S // P            # 16 token tiles
ND = D // P            # 24 contraction chunks
HL = 3                 # heads per core
HD = 128
FQ = HL * HD           # 384 local q (or k, or v) features
NCORES = 8
EPS = 1e-5

F16 = mybir.dt.float16
F32 = mybir.dt.float32
AF = mybir.ActivationFunctionType
ALU = mybir.AluOpType


def _patch_tile_drain():
    """This walrus build allows only one sync-wait on a Drain CTRL
    instruction; split the final TileContext drain's waits across
    single-wait nops."""
    if getattr(tile.TileContext, '_drain_patched', False):
        return

    def _drain_and_barrier(self, tick_clock, wait_clock):
        nc = self.nc
        drain_inst = nc.sync.drain()
        wait_clock.add_sem_waits(
            drain_inst.ins, ScopedClock({None: tick_clock.global_clock})
        )
        si = drain_inst.ins.sync_info
        waits = list(si.on_wait or [])
        if len(waits) > 1:
            si.on_wait = waits[:1]
            for w in waits[1:]:
                nop = nc.sync.nop(nofuse=True)
                nop.ins.sync_info = mybir.SyncInfo(on_wait=[w], on_update=[])
        nc.all_engine_barrier()
        assert self.sems is not None
        popped = nc._tile_sem_poison_stack.pop()
        assert popped is self._sem_poison
        nc.clear_and_free_semaphores(list(self.sems.allocated().values()))
        nc.all_engine_barrier()

    tile.TileContext._drain_and_barrier = _drain_and_barrier
    tile.TileContext._drain_patched = True


def _split_excess_waits(nc, cap=1):
    """This walrus build supports very few sync-wait slots per instruction.
    Hoist excess waits onto preceding single-wait nops on the same engine."""
    uid = [0]
    for fn in nc.m.functions:
        for bb in fn.blocks:
            out = []
            changed = False
            for inst in bb.instructions:
                si = getattr(inst, 'sync_info', None)
                waits = list(si.on_wait) if si is not None and si.on_wait else []
                if len(waits) > cap and inst.engine is not None:
                    changed = True
                    for w in waits[:-cap]:
                        uid[0] += 1
                        out.append(mybir.InstNoOp(
                            name=f"wsplit_{uid[0]}",
                            sync_info=mybir.SyncInfo(on_wait=[w], on_update=[]),
                            bass_nofuse=True,
                            engine=inst.engine,
                        ))
                    si.on_wait = waits[-cap:]
                out.append(inst)
            if changed:
                bb.instructions = out


def _swap_pairs(ap):
    """View of [..., 2k] AP with adjacent element pairs swapped; returns a
    4-d [P, HL, HD//2, 2] view."""
    return ap.rearrange("p h (a b) -> p h a b", b=2)[:, :, :, ::-1]


def _pairs(ap):
    return ap.rearrange("p h (a b) -> p h a b", b=2)


def build_bass():
    _patch_tile_drain()
    nc = bass.Bass()

    # DRAM I/O (all pre-laid-out on host to match SBUF tiles).
    xt = nc.dram_tensor("xt", [NT, P, ND, P], F16, kind="ExternalInput")
    wqkv_t = nc.dram_tensor("wqkv_t", [P, ND, 3 * FQ], F16, kind="ExternalInput")
    wqkv_i = nc.dram_tensor("wqkv_i", [P, ND, 3 * FQ], F16, kind="ExternalInput")
    wout_t = nc.dram_tensor("wout_t", [P, HL, D], F16, kind="ExternalInput")
    wout_i = nc.dram_tensor("wout_i", [P, HL, D], F16, kind="ExternalInput")
    caq = nc.dram_tensor("caq", [P, NT, HD], F16, kind="ExternalInput")
    saq = nc.dram_tensor("saq", [P, NT, HD], F16, kind="ExternalInput")
    cak = nc.dram_tensor("cak", [P, NT, HD], F16, kind="ExternalInput")
    sak = nc.dram_tensor("sak", [P, NT, HD], F16, kind="ExternalInput")
    ident = nc.dram_tensor("ident", [P, P], F16, kind="ExternalInput")
    ones_col = nc.dram_tensor("ones_col", [P, 1], F16, kind="ExternalInput")
    ones_row = nc.dram_tensor("ones_row", [1, P], F16, kind="ExternalInput")
    epsb = nc.dram_tensor("epsb", [P, 1], F32, kind="ExternalInput")
    y = nc.dram_tensor("y", [S, D], F16, kind="ExternalOutput")

    with tile.TileContext(nc) as tc, ExitStack() as ctx:
        const = ctx.enter_context(tc.tile_pool(name="const", bufs=1))
        persist = ctx.enter_context(tc.tile_pool(name="persist", bufs=1))
        np2 = ctx.enter_context(tc.tile_pool(name="np2", bufs=2))

        ident_sb = const.tile([P, P], F16, tag="ident")
        onec_sb = const.tile([P, 1], F16, tag="onec")
        oner_sb = const.tile([1, P], F16, tag="oner")
        eps_sb = const.tile([P, 1], F32, tag="epsb")
        tabs = {}
        for name in ("caq", "saq", "cak", "sak"):
            tabs[name] = const.tile([P, NT, HD], F16, tag=name, name=name)

        # persistent activations; attnT split per q-tile group so the out
        # projection can start as soon as a group's softmax is finalized
        qT = persist.tile([P, HL, S], F16, tag="qT")
        kT = persist.tile([P, HL, S], F16, tag="kT")
        v_sb = persist.tile([P, NT, FQ], F16, tag="v")
        NQT = 4
        QW = S // NQT    # 512
        attnT = [persist.tile([P, HL, QW], F16, tag=f"attnT{g}",
                              name=f"attnT{g}")
                 for g in range(NQT)]
        # tile-15 q/k psums are evacuated to SBUF so their norm/rope/transpose
        # pipelines can run inside phase 2 (removes the phase-1 tail)
        q15_sb = persist.tile([P, FQ], F32, tag="q15")
        k15_sb = persist.tile([P, FQ], F32, tag="k15")

        def qk_pipeline(src3, ca_t, sa_t, dstT, i, ps_pool, ps_tag,
                        sq_pool=None):
            """RMSNorm + RoPE + PE-transpose for one [P, HL, HD] q/k tile.
            src3 may live in PSUM or SBUF (fp32). dve_norm routes the norm
            arithmetic to DVE (used for the deferred tile-15 pipelines so
            they don't delay the phase-2 exp stream on ACT)."""
            dve_norm = sq_pool is not None
            ms = np2.tile([P, HL], F32, tag="ms")
            if dve_norm:
                sq = sq_pool.tile([P, HL, HD], F32, tag="sq15")
                nc.vector.tensor_tensor(sq[:], src3[:], src3[:], ALU.mult)
                nc.vector.tensor_reduce(ms[:], sq[:], axis=mybir.AxisListType.X,
                                        op=ALU.add)
            else:
                for h in range(HL):
                    scr = np2.tile([P, HD], F32, tag="sqscr")
                    nc.scalar.activation(scr[:], src3[:, h, :], AF.Square,
                                         accum_out=ms[:, h:h + 1])
            srt = np2.tile([P, HL], F32, tag="srt")
            nc.scalar.activation(srt[:], ms[:], AF.Sqrt,
                                 scale=1.0 / HD, bias=eps_sb[:])
            s = np2.tile([P, HL], F32, tag="s")
            nc.vector.reciprocal(s[:], srt[:])
            qn = np2.tile([P, HL, HD], F16, tag="qn")
            if dve_norm:
                nc.vector.tensor_tensor(
                    qn[:], src3[:],
                    s[:, :, None].to_broadcast([P, HL, HD]), ALU.mult)
            else:
                for h in range(HL):
                    nc.scalar.mul(qn[:, h, :], src3[:, h, :], s[:, h:h + 1])
            # rope: o = qn*ca + swap(qn)*sa
            ca_b = ca_t[:, i, :].unsqueeze(1).to_broadcast([P, HL, HD])
            sa_b = sa_t[:, i, :].unsqueeze(1).to_broadcast([P, HL, HD])
            ra = np2.tile([P, HL, HD], F16, tag="ra")
            nc.vector.tensor_tensor(ra[:], qn[:], ca_b, ALU.mult)
            rb = np2.tile([P, HL, HD], F16, tag="rb")
            nc.vector.tensor_tensor(_pairs(rb), _swap_pairs(qn),
                                    _pairs(sa_b), ALU.mult)
            qr = np2.tile([P, HL, HD], F16, tag="qr")
            nc.vector.tensor_tensor(qr[:], ra[:], rb[:], ALU.add)
            for h in range(HL):
                ps_t = ps_pool.tile([P, P], F32, tag=ps_tag,
                                    name=f"tr_{i}_{h}")
                nc.tensor.matmul(ps_t[:], lhsT=qr[:, h, :], rhs=ident_sb[:],
                                 start=True, stop=True)
                nc.vector.tensor_copy(dstT[:, h, ts(i, P)], ps_t[:])

        # ---------------- Phase 1: QKV + norm + rope + transpose ----------
        with tc.tile_pool(name="wqkv", bufs=1) as wpool, \
             tc.tile_pool(name="xp", bufs=4) as xpool, \
             tc.tile_pool(name="ps_qkv", bufs=2, space="PSUM") as psum_qkv, \
             tc.tile_pool(name="ps_tr", bufs=2, space="PSUM") as psum_tr:

            # DMA emission order tuned for the serial ~0.6us/trigger on the
            # sync sequencer and ~400 GB/s aggregate bandwidth: xt tile 0,
            # first weight chunks, rope tables mid-stream, remaining weights
            w_txt_sb = wpool.tile([P, ND, 3 * FQ], F16, tag="wt")
            w_img_sb = wpool.tile([P, ND, 3 * FQ], F16, tag="wi")

            def load_xt(i):
                strip = xpool.tile([P, ND, P], F16, tag="xt", name=f"xt{i}")
                nc.sync.dma_start(strip[:], xt[i])
                return strip

            xt_strips = [None] * NT
            xt_strips[0] = load_xt(0)
            for j in range(0, 4):
                nc.sync.dma_start(w_txt_sb[:, j, :], wqkv_t[:, j, :])
            xt_strips[1] = load_xt(1)
            for j in range(4, 12, 2):
                nc.sync.dma_start(w_txt_sb[:, j:j + 2, :], wqkv_t[:, j:j + 2, :])
            xt_strips[2] = load_xt(2)
            for name, dram in (("caq", caq), ("saq", saq),
                               ("cak", cak), ("sak", sak)):
                nc.sync.dma_start(tabs[name][:], dram[:])
            for j in range(12, 18, 2):
                nc.sync.dma_start(w_txt_sb[:, j:j + 2, :], wqkv_t[:, j:j + 2, :])
            xt_strips[3] = load_xt(3)
            for j in range(18, ND, 2):
                nc.sync.dma_start(w_txt_sb[:, j:j + 2, :], wqkv_t[:, j:j + 2, :])
            nc.sync.dma_start(ident_sb[:], ident[:])
            nc.sync.dma_start(onec_sb[:], ones_col[:])
            nc.sync.dma_start(oner_sb[:], ones_row[:])
            nc.sync.dma_start(eps_sb[:], epsb[:])
            for j in range(0, ND, 2):
                nc.sync.dma_start(w_img_sb[:, j:j + 2, :], wqkv_i[:, j:j + 2, :])

            for i in range(NT):
                w_sb = w_txt_sb if i < S_TXT // P else w_img_sb
                if xt_strips[i] is None:
                    xt_strips[i] = load_xt(i)
                if i + 3 < NT and xt_strips[i + 3] is None:
                    xt_strips[i + 3] = load_xt(i + 3)
                xt_strip = xt_strips[i]

                psq = psum_qkv.tile([P, FQ], F32, tag="psq")
                psk = psum_qkv.tile([P, FQ], F32, tag="psk")
                psv = psum_qkv.tile([P, FQ], F32, tag="psv")
                for j in range(ND):
                    lh = xt_strip[:, j, :]
                    st = (j == 0)
                    sp = (j == ND - 1)
                    nc.tensor.matmul(psq[:], lhsT=lh, rhs=w_sb[:, j, 0:FQ],
                                     start=st, stop=sp)
                    nc.tensor.matmul(psk[:], lhsT=lh, rhs=w_sb[:, j, FQ:2 * FQ],
                                     start=st, stop=sp)
                    nc.tensor.matmul(psv[:], lhsT=lh, rhs=w_sb[:, j, 2 * FQ:3 * FQ],
                                     start=st, stop=sp)

                nc.vector.tensor_copy(v_sb[:, i, :], psv[:])

                if i == NT - 1:
                    # defer tile-15 q/k processing into phase 2
                    nc.vector.tensor_copy(q15_sb[:], psq[:])
                    nc.vector.tensor_copy(k15_sb[:], psk[:])
                else:
                    qk_pipeline(psq.rearrange("p (h d) -> p h d", d=HD),
                                tabs["caq"], tabs["saq"], qT, i,
                                psum_tr, "tr")
                    qk_pipeline(psk.rearrange("p (h d) -> p h d", d=HD),
                                tabs["cak"], tabs["sak"], kT, i,
                                psum_tr, "tr")

        # ---------------- Phase 2: attention -----------------------------
        # qt-outer; denominator finalize staged at hook points one iteration
        # later so no in-order engine queue blocks. Out-projection weights
        # prefetched here (phase-1 weight space was just freed).
        wopool = ctx.enter_context(tc.tile_pool(name="wout", bufs=1))
        wo_txt_sb = wopool.tile([P, HL, D], F16, tag="wot")
        wo_img_sb = wopool.tile([P, HL, D], F16, tag="woi")
        for ch in range(HL):
            nc.sync.dma_start(wo_txt_sb[:, ch, :], wout_t[:, ch, :])
        for ch in range(HL):
            nc.sync.dma_start(wo_img_sb[:, ch, :], wout_i[:, ch, :])

        with tc.tile_pool(name="pp", bufs=6) as ppool, \
             tc.tile_pool(name="tree", bufs=24) as tpool, \
             tc.tile_pool(name="rp", bufs=2) as rpool, \
             tc.tile_pool(name="ps_s", bufs=2, space="PSUM") as psum_s, \
             tc.tile_pool(name="ps_u", bufs=2, space="PSUM") as psum_u, \
             tc.tile_pool(name="ps_d", bufs=1, space="PSUM") as psum_d, \
             tc.tile_pool(name="ps_r", bufs=1, space="PSUM") as psum_r:

            def attn_compute(h, qt, hooks=()):
                # hooks: {ck2_group: callable} run after that group's emission
                ps_u = psum_u.tile([P, QW], F32, tag="u")
                lvl = []
                hooks = dict(hooks)
                for ck2 in range(NT // 2):
                    ps_s = psum_s.tile([P, 2 * QW], F32, tag="s")
                    for half in range(2):
                        ck = 2 * ck2 + half
                        nc.tensor.matmul(
                            ps_s[:, half * QW:(half + 1) * QW],
                            lhsT=kT[:, h, ts(ck, P)],
                            rhs=qT[:, h, ts(qt, QW)],
                            start=True, stop=True)
                    p2 = ppool.tile([P, 2 * QW], F16, tag="p2")
                    nc.scalar.activation(p2[:], ps_s[:], AF.Exp)
                    for half in range(2):
                        ck = 2 * ck2 + half
                        nc.tensor.matmul(
                            ps_u[:],
                            lhsT=v_sb[:, ck, ts(h, HD)],
                            rhs=p2[:, half * QW:(half + 1) * QW],
                            start=(ck == 0), stop=(ck == NT - 1))
                    t = tpool.tile([P, QW], F16, tag="tree")
                    # pair-sum each exp tile's halves (alternating DVE/GpSimd,
                    # last group on DVE so the tail chain stays short), then
                    # fold into a running sum on DVE
                    eng = nc.vector if (ck2 % 2 == 0 or ck2 == NT // 2 - 1) \
                        else nc.gpsimd
                    eng.tensor_tensor(t[:], p2[:, 0:QW], p2[:, QW:2 * QW],
                                      ALU.add)
                    if lvl:
                        r = tpool.tile([P, QW], F16, tag="tree")
                        nc.vector.tensor_tensor(r[:], lvl[-1][:], t[:], ALU.add)
                        lvl.append(r)
                    else:
                        lvl.append(t)
                    if ck2 in hooks:
                        hooks[ck2]()
                return ps_u, lvl[-1]

            # denominator finalize for iteration k-1, staged inside iteration
            # k: d-matmul, then 1/d = exp(-ln d) on ACT (same table set as the
            # softmax exp), then the broadcast matmul + normalizing multiply
            def make_hooks(prev):
                ph, pqt, pps_u, pdsum = prev
                st = {}

                def s0():
                    ps_d = psum_d.tile([1, QW], F32, tag="d")
                    nc.tensor.matmul(ps_d[:], lhsT=onec_sb[:], rhs=pdsum[:],
                                     start=True, stop=True)
                    st['ps_d'] = ps_d

                def s1():
                    lnd = rpool.tile([1, QW], F32, tag="lnd")
                    nc.scalar.activation(lnd[:], st['ps_d'][:], AF.Ln)
                    rcp16 = rpool.tile([1, QW], F16, tag="rcp16")
                    nc.scalar.activation(rcp16[:], lnd[:], AF.Exp, scale=-1.0)
                    st['rcp16'] = rcp16

                def s2():
                    ps_rep = psum_r.tile([P, QW], F32, tag="rep")
                    nc.tensor.matmul(ps_rep[:], lhsT=oner_sb[:],
                                     rhs=st['rcp16'][:], start=True, stop=True)
                    rep16 = rpool.tile([P, QW], F16, tag="rep16")
                    nc.vector.tensor_copy(rep16[:], ps_rep[:])
                    nc.vector.tensor_tensor(attnT[pqt][:, ph, :], pps_u[:],
                                            rep16[:], ALU.mult)

                return {1: s0, 2: s1, 4: s2}

            prev = None
            for qt in range(NQT):
                for h in range(HL):
                    k = qt * HL + h
                    if prev is None:
                        # first iteration: emit the deferred tile-15 k
                        # pipeline (needed by this iteration's last group)
                        hooks = {
                            0: lambda: qk_pipeline(
                                k15_sb.rearrange("p (h d) -> p h d", d=HD),
                                tabs["cak"], tabs["sak"], kT, NT - 1,
                                psum_d, "d", sq_pool=rpool),
                        }
                    else:
                        hooks = make_hooks(prev)
                        if k == 1:
                            # deferred tile-15 q pipeline (needed from qt=3)
                            hooks[6] = lambda: qk_pipeline(
                                q15_sb.rearrange("p (h d) -> p h d", d=HD),
                                tabs["caq"], tabs["saq"], qT, NT - 1,
                                psum_d, "d", sq_pool=rpool)
                    prev = (h, qt, *attn_compute(h, qt, hooks))
            # evacuate the last iteration's psum/tree results to SBUF so the
            # tail finalize can run in phase 3 after these pools close
            th, tqt, tps_u, tdsum = prev
            u_tail_sb = wopool.tile([P, QW], F32, tag="u_tail")
            nc.vector.tensor_copy(u_tail_sb[:], tps_u[:])
            dsum_tail_sb = wopool.tile([P, QW], F16, tag="dsum_tail")
            nc.vector.tensor_copy(dsum_tail_sb[:], tdsum[:])
            tail = (th, tqt, u_tail_sb, dsum_tail_sb)

        # ---------------- Phase 3: out projection -------------------------
        YW = 1536
        with tc.tile_pool(name="yp", bufs=3) as ypool, \
             tc.tile_pool(name="ps_y", bufs=2, space="PSUM") as psum_y:
            th, tqt, tps_u, tdsum = tail
            tst = {}

            def tail_s0():
                ps_d = psum_y.tile([1, QW], F32, tag="y", name="tail_ps_d")
                nc.tensor.matmul(ps_d[:], lhsT=onec_sb[:], rhs=tdsum[:],
                                 start=True, stop=True)
                lnd = ypool.tile([1, QW], F32, tag="lnd_t", name="tail_lnd")
                nc.scalar.activation(lnd[:], ps_d[:], AF.Ln)
                rcp16 = ypool.tile([1, QW], F16, tag="rcp16_t",
                                    name="tail_rcp16")
                nc.scalar.activation(rcp16[:], lnd[:], AF.Exp, scale=-1.0)
                tst['rcp16'] = rcp16

            def tail_s1():
                ps_rep = psum_y.tile([P, QW], F32, tag="y", name="tail_ps_rep")
                nc.tensor.matmul(ps_rep[:], lhsT=oner_sb[:],
                                 rhs=tst['rcp16'][:], start=True, stop=True)
                rep16 = ypool.tile([P, QW], F16, tag="rep16_t",
                                    name="tail_rep16")
                nc.vector.tensor_copy(rep16[:], ps_rep[:])
                nc.vector.tensor_tensor(attnT[tqt][:, th, :], tps_u[:],
                                        rep16[:], ALU.mult)

            for i in range(NT):
                w_sb = wo_txt_sb if i < S_TXT // P else wo_img_sb
                g, r = i // (QW // P), i % (QW // P)
                for half in range(2):
                    ps_y = psum_y.tile([P, YW], F32, tag="y")
                    for ch in range(HL):
                        lh = attnT[g][:, ch, ts(r, P)]
                        for dd in range(3):
                            nc.tensor.matmul(
                                ps_y[:, dd * QW:(dd + 1) * QW],
                                lhsT=lh,
                                rhs=w_sb[:, ch, half * YW + dd * QW:
                                         half * YW + (dd + 1) * QW],
                                start=(ch == 0), stop=(ch == HL - 1))
                    y_t = ypool.tile([P, YW], F16, tag="yt")
                    nc.scalar.copy(y_t[:], ps_y[:])
                    nc.sync.dma_start(
                        y[ts(i, P), half * YW:(half + 1) * YW], y_t[:])
                # the last attention iteration's finalize is interleaved here
                # (tile 12-15 are the only consumers of its attnT group)
                if i == 0:
                    tail_s0()
                elif i == 1:
                    tail_s1()

    _split_excess_waits(nc)
    return nc


def _host_prep(inputs):
    f = lambda n: np.asarray(inputs[n], dtype=np.float32)
    hs = f('hidden_states')[0]
    ehs = f('encoder_hidden_states')[0]
    X = np.concatenate([ehs, hs], axis=0)              # [2048, 3072] txt|img
    XT = np.ascontiguousarray(X.T)                     # [3072, 2048]
    # [NT, P(tok), ND, P(d)] tiled layout matching SBUF strips
    xt_tiled = np.ascontiguousarray(
        XT.reshape(ND, P, NT, P).transpose(2, 1, 0, 3)).astype(np.float16)

    Wqkv_i = f('qkv_w').T + f('qkv_down') @ f('qkv_up')          # [3072, 9216]
    Wqkv_t = f('add_qkv_w').T + f('add_qkv_down') @ f('add_qkv_up')
    Wo_i = f('out_w').T + f('out_down') @ f('out_up')            # [3072, 3072]
    Wo_t = f('add_out_w').T + f('add_out_down') @ f('add_out_up')

    cos, sin = f('rope_cos'), f('rope_sin')            # [2048, 64]
    C2 = np.repeat(cos, 2, axis=1)                     # [2048, 128]
    S2 = np.repeat(sin, 2, axis=1)
    S2[:, 0::2] *= -1.0

    def tok_norm_w(w_img, w_txt):
        w = np.empty((S, HD), np.float32)
        w[:S_TXT] = w_txt
        w[S_TXT:] = w_img
        return w

    def swap(w):
        o = np.empty_like(w)
        o[:, 0::2] = w[:, 1::2]
        o[:, 1::2] = w[:, 0::2]
        return o

    wq = tok_norm_w(f('norm_q_w'), f('norm_added_q_w'))
    wk = tok_norm_w(f('norm_k_w'), f('norm_added_k_w'))
    qscale = 1.0 / np.sqrt(np.float32(HD))
    CA_q = (wq * C2) * qscale
    SA_q = (swap(wq) * S2) * qscale
    CA_k = wk * C2
    SA_k = swap(wk) * S2

    def tab16(t):  # [2048, 128] -> [P, NT, HD]
        return np.ascontiguousarray(
            t.reshape(NT, P, HD).transpose(1, 0, 2)).astype(np.float16)

    def wqkv16(W):  # [3072, 9216] core slice -> [P, ND, 3*FQ]
        def core_slice(W, c):
            qs = slice(c * FQ, (c + 1) * FQ)
            ks = slice(D + c * FQ, D + (c + 1) * FQ)
            vs = slice(2 * D + c * FQ, 2 * D + (c + 1) * FQ)
            return np.concatenate([W[:, qs], W[:, ks], W[:, vs]], axis=1)
        return [np.ascontiguousarray(
            core_slice(W, c).reshape(ND, P, 3 * FQ).transpose(1, 0, 2)
        ).astype(np.float16) for c in range(NCORES)]

    def wout16(W):  # [3072, 3072] row slice per core -> [P, HL, D]
        return [np.ascontiguousarray(
            W[c * FQ:(c + 1) * FQ, :].reshape(HL, P, D).transpose(1, 0, 2)
        ).astype(np.float16) for c in range(NCORES)]

    shared = {
        'xt': xt_tiled,
        'epsb': np.full((P, 1), EPS, np.float32),
        'caq': tab16(CA_q), 'saq': tab16(SA_q),
        'cak': tab16(CA_k), 'sak': tab16(SA_k),
        'ident': np.eye(P, dtype=np.float16),
        'ones_col': np.ones((P, 1), np.float16),
        'ones_row': np.ones((1, P), np.float16),
    }
    wqkv_i_c = wqkv16(Wqkv_i)
    wqkv_t_c = wqkv16(Wqkv_t)
    wo_i_c = wout16(Wo_i)
    wo_t_c = wout16(Wo_t)
    in_maps = []
    for c in range(NCORES):
        m = dict(shared)
        m['wqkv_i'] = wqkv_i_c[c]
        m['wqkv_t'] = wqkv_t_c[c]
        m['wout_i'] = wo_i_c[c]
        m['wout_t'] = wo_t_c[c]
        in_maps.append(m)
    return in_maps


_NC_CACHE = None


def run_device(inputs, trace=False, **kw):
    global _NC_CACHE
    from concourse.bass_utils import run_bass_kernel_spmd
    if _NC_CACHE is None:
        _NC_CACHE = build_bass()
    in_maps = _host_prep(inputs)
    return run_bass_kernel_spmd(_NC_CACHE, in_maps, core_ids=list(range(NCORES)),
                                trace=trace, **kw)


def kernel(**inputs):
    res = run_device(inputs, trace=False)
    Y = np.zeros((S, D), np.float32)
    for r in res.results:
        Y += r['y'].astype(np.float32)
    out_b = np.asarray(inputs['out_b'], dtype=np.float32)
    add_out_b = np.asarray(inputs['add_out_b'], dtype=np.float32)
    img = (Y[S_TXT:] + out_b)[None].astype(np.float32)
    txt = (Y[:S_TXT] + add_out_b)[None].astype(np.float32)
    return (img, txt)
